# revision 40
# baseline (speedup 1.0000x reference)
"""Trainium2 Bass kernel for nn_Attention_3375844294750.

Cross-attention (q from x, k/v from context) with key mask, 8 heads, d=64.
  B=4, N=M=2048, query_dim=context_dim=512, inner=512.

Sharding: 8 NeuronCores = (batch b = core//2) x (query-half = core%2).
Each core computes attention for its 1024 queries over its batch's keys.
No collectives needed (outputs are disjoint).

Key compaction: masked keys contribute exactly 0 to masked softmax, so the
CPU glue gathers only the unmasked keys (~50% of 2048) per batch, padded
to a multiple of 128; padding slots are killed by the exp bias. This
halves the score/exp/PV work.

Structure: the attention inner loop streams score->exp->PV per m-tile,
with the PV stream lagged two m-tiles behind the scores so the in-order
PE queue never waits on the ACT(exp) stream and the previous block's PSUM
accumulators have time to drain before reuse. Projection and
output-projection units are dripped into the PE slack under the exp
stream, one unit per iteration.

Per-core math (all matmuls bf16 with fp32 PSUM accumulation):
  qT = (x @ Wq)^T        [inner, n]   via rhs = x^T (CPU pre-transposed)
  kT = (ctx_c @ Wk)^T    [inner, m_c]
  v  = ctx_c @ Wv        [m_c, inner] (+ ones column per head for the
                                       softmax denominator)
  S^T = kT_h-blocks @ qT_h            [m_c, n] per head-pair, K=64
  P^T = exp(S*scale + pad_bias)       one-pass softmax (logits bounded,
                                      no max subtraction needed)
  O^T_h (+denom row) = [V_h|1]^T @ P^T_h   accumulated over m-tiles
  O_norm^T = O^T * (1/denom): the denominator rows are partition-broadcast
  via a DRAM bounce (hidden under the next block) except for the final
  block, which instead casts them to bf16 and broadcasts on the then-idle
  PE, and whose output projection takes the second head half directly
  from the normalize result via a split-K accumulation (no shift DMA) —
  keeping the drain chain short.
  out = O_norm^T-blocks^T @ Wo + bo   (SBUF-accumulated per head-pair)

Startup: inputs load as a few batched multi-dim DMAs split across the two
hardware DGE queues (sync + scalar), staged so the first projection units
wait on the least data; throwaway matmuls ramp the PE clock and a
throwaway exp pulls the activation-table load off the critical path while
the first tiles land.
"""
import os
import sys

for _p in ("/opt/trn_rl_repo", "/root/.axon_site/_ro/trn_rl_repo"):
    if os.path.isdir(_p) and _p not in sys.path:
        sys.path.insert(0, _p)
        break

import numpy as np
import ml_dtypes

B, N, M = 4, 2048, 2048
QD = 512          # query_dim == context_dim
H, D = 8, 64
INNER = H * D     # 512
SCALE = D ** -0.5
NCORE = N // 2    # queries per core = 1024
P = 128
NBLK = 512        # n-block (one PSUM bank per matmul)
MASK_NEG = -1e30

_CACHE = {}


def _build_nc(nmt):
    """Build + compile the SPMD program for nmt m-tiles (m_pad = 128*nmt)."""
    import concourse.mybir as mybir
    from concourse import bacc
    from concourse.tile import TileContext
    import concourse.bass as bass

    mpad = nmt * P
    dt = mybir.dt
    nc = bacc.Bacc("TRN2", target_bir_lowering=False, debug=False, num_devices=8)

    xT_d = nc.declare_dram_parameter("xT", [4, P, NCORE], dt.bfloat16, isOutput=False)
    ctxT_d = nc.declare_dram_parameter("ctxT", [4, P, mpad], dt.bfloat16, isOutput=False)
    wq_d = nc.declare_dram_parameter("wq", [4, P, INNER], dt.bfloat16, isOutput=False)
    wk_d = nc.declare_dram_parameter("wk", [4, P, INNER], dt.bfloat16, isOutput=False)
    wv_d = nc.declare_dram_parameter("wv", [4, P, INNER], dt.bfloat16, isOutput=False)
    wo_d = nc.declare_dram_parameter("wo", [4, P, QD], dt.bfloat16, isOutput=False)
    bo_d = nc.declare_dram_parameter("bo", [1, QD], dt.float32, isOutput=False)
    mb_d = nc.declare_dram_parameter("mb", [P, nmt], dt.float32, isOutput=False)
    out_d = nc.declare_dram_parameter("out", [NCORE, QD], dt.float32, isOutput=True)

    f32 = dt.float32
    bf16 = dt.bfloat16
    EXP = mybir.ActivationFunctionType.Exp

    with TileContext(nc) as tc:
        from contextlib import ExitStack

        with ExitStack() as ctx:
            const = ctx.enter_context(tc.tile_pool(name="const", bufs=1))

            # ---- persistent SBUF tensors ----
            wq_sb = const.tile([P, 4, INNER], bf16, tag="wq")
            xT_sb = const.tile([P, 4, NCORE], bf16, tag="xT")
            wk_sb = const.tile([P, 4, INNER], bf16, tag="wk")
            ctxT_sb = const.tile([P, 4, mpad], bf16, tag="cT")
            wv_sb = const.tile([P, 4, INNER], bf16, tag="wv")
            wo_sb = const.tile([P, 4, QD], bf16, tag="wo")
            bo_bc = const.tile([P, QD], f32, tag="bo")
            mb_sb = const.tile([P, nmt], f32, tag="mb")
            wm_sb = const.tile([P, NBLK], bf16, tag="wm")

            ones_sb = const.tile([P, D], bf16, tag="ones")
            dm_sb = const.tile([P, 8], bf16, tag="dm")
            wo3b = const.tile([D, QD], bf16, tag="wo3b")
            qT_sb = const.tile([P, 4, NCORE], bf16, tag="qT")
            kT_sb = const.tile([P, 4, mpad], bf16, tag="kT")
            v_sb = const.tile([P, nmt, H, D + 1], bf16, tag="v")
            o_sb = const.tile([P, 4, NCORE], bf16, tag="oT")
            fin_sb = const.tile([P, NCORE // P, QD], f32, tag="fin")

            # warm-up weights for the PE clock ramp (never read by output)
            nc.vector.memset(wm_sb[:], 1.0)
            nc.vector.memset(ones_sb[:], 1.0)

            # ---- input loads: batched multi-dim DMAs over two issue
            # queues, staged so the first projection units wait on the
            # least possible data (slice 0 + left column halves first)
            def dbox(t, s0, ns, p0, np_, c0, nc_):
                a = t.ap()
                S, Pp, C = a.ap[0][0], a.ap[1][0], a.ap[2][0]
                return bass.AP(
                    tensor=a.tensor,
                    offset=a.offset + s0 * S + p0 * Pp + c0 * C,
                    ap=[[Pp, np_], [S, ns], [C, nc_]])

            nc.sync.dma_start(out=wq_sb[:, 0:1, :],
                              in_=dbox(wq_d, 0, 1, 0, P, 0, INNER))
            nc.sync.dma_start(out=xT_sb[:, 0:1, 0:NBLK],
                              in_=dbox(xT_d, 0, 1, 0, P, 0, NBLK))
            nc.scalar.dma_start(out=ctxT_sb[:, :, 0:NBLK],
                                in_=dbox(ctxT_d, 0, 4, 0, P, 0, NBLK))
            nc.sync.dma_start(out=wq_sb[:, 1:4, :],
                              in_=dbox(wq_d, 1, 3, 0, P, 0, INNER))
            nc.sync.dma_start(out=xT_sb[:, 1:4, 0:NBLK],
                              in_=dbox(xT_d, 1, 3, 0, P, 0, NBLK))
            nc.scalar.dma_start(out=xT_sb[:, :, NBLK:NCORE],
                                in_=dbox(xT_d, 0, 4, 0, P, NBLK, NCORE - NBLK))
            nc.sync.dma_start(out=wk_sb[:],
                              in_=dbox(wk_d, 0, 4, 0, P, 0, INNER))
            nc.scalar.dma_start(out=ctxT_sb[:, :, NBLK:mpad],
                                in_=dbox(ctxT_d, 0, 4, 0, P, NBLK, mpad - NBLK))
            nc.sync.dma_start(out=wv_sb[:],
                              in_=dbox(wv_d, 0, 4, 0, P, 0, INNER))
            nc.scalar.dma_start(out=wo_sb[:],
                                in_=dbox(wo_d, 0, 4, 0, P, 0, QD))
            nc.sync.dma_start(out=mb_sb[:], in_=mb_d[:])
            nc.sync.dma_start(out=wo3b[:], in_=dbox(wo_d, 3, 1, D, D, 0, QD))
            bo_src = bass.AP(tensor=bo_d.ap().tensor, offset=bo_d.ap().offset,
                             ap=[[0, P]] + bo_d.ap().ap[1:])
            nc.sync.dma_start(out=bo_bc[:], in_=bo_src)

            # ones columns for the denominator trick (copies below leave them)
            nc.vector.memset(v_sb[:, :, :, D], 1.0)

            mchunks = []
            off = 0
            while off < mpad:
                w = min(NBLK, mpad - off)
                mchunks.append((off, w))
                off += w

            with tc.tile_pool(name="aux", bufs=2, space="PSUM") as aux, \
                 tc.tile_pool(name="sps", bufs=2, space="PSUM") as sps, \
                 tc.tile_pool(name="ops", bufs=1, space="PSUM") as ops, \
                 tc.tile_pool(name="ppool", bufs=6) as ppool, \
                 tc.tile_pool(name="raw", bufs=4) as rawp, \
                 tc.tile_pool(name="bcp", bufs=2) as bcp, \
                 tc.tile_pool(name="dscr", bufs=4, space="DRAM") as dscr:

                # a few throwaway matmuls ramp the PE p-state while the
                # first input tiles are still in flight, and a throwaway
                # exp pulls the activation table load off the critical path
                for _ in range(4):
                    pw = aux.tile([P, NBLK], f32, tag="aux", name="warm")
                    nc.tensor.matmul(pw[:], lhsT=wm_sb[:, 0:P],
                                     rhs=wm_sb[:], start=True, stop=True)
                nc.scalar.activation(out=dm_sb[0:1, :], in_=wm_sb[0:1, 0:8],
                                     func=EXP, scale=1.0)

                def v_unit(mt):
                    def f():
                        ps = aux.tile([P, INNER], f32, tag="aux", name="psv")
                        for kq in range(4):
                            nc.tensor.matmul(
                                ps[:],
                                lhsT=ctxT_sb[:, kq, mt * P:(mt + 1) * P],
                                rhs=wv_sb[:, kq, :],
                                start=(kq == 0), stop=(kq == 3),
                            )
                        psh = ps.rearrange("p (h d) -> p h d", h=H)
                        nc.vector.tensor_copy(v_sb[:, mt, :, 0:D], psh[:])
                    return f

                # deferred aux work (projection slices, output-proj
                # units) dripped into the attention stream so the exp
                # pipeline never starves
                pending = []

                def proj_unit_q(mi, nh):
                    def f():
                        ps = aux.tile([P, NBLK], f32, tag="aux", name="psq")
                        for kq in range(4):
                            nc.tensor.matmul(
                                ps[:],
                                lhsT=wq_sb[:, kq, mi * P:(mi + 1) * P],
                                rhs=xT_sb[:, kq, nh * NBLK:(nh + 1) * NBLK],
                                start=(kq == 0), stop=(kq == 3),
                            )
                        nc.vector.tensor_copy(
                            qT_sb[:, mi, nh * NBLK:(nh + 1) * NBLK], ps[:])
                    return f

                def proj_unit_k(mi, off, w):
                    def f():
                        ps = aux.tile([P, NBLK], f32, tag="aux", name="psk")
                        for kq in range(4):
                            nc.tensor.matmul(
                                ps[:, 0:w],
                                lhsT=wk_sb[:, kq, mi * P:(mi + 1) * P],
                                rhs=ctxT_sb[:, kq, off:off + w],
                                start=(kq == 0), stop=(kq == 3),
                            )
                        nc.vector.tensor_copy(
                            kT_sb[:, mi, off:off + w], ps[:, 0:w])
                    return f

                def fin_unit(p, nt):
                    def f():
                        ps = aux.tile([P, NBLK], f32, tag="aux", name="psf")
                        nc.tensor.matmul(
                            ps[:, 0:QD],
                            lhsT=o_sb[:, p, nt * P:(nt + 1) * P],
                            rhs=wo_sb[:, p, :],
                            start=True, stop=True,
                        )
                        if p == 0:
                            nc.vector.tensor_add(
                                fin_sb[:, nt, :], ps[:, 0:QD], bo_bc[:])
                        else:
                            nc.vector.tensor_add(
                                fin_sb[:, nt, :], ps[:, 0:QD],
                                fin_sb[:, nt, :])
                        if p == 3:
                            nc.sync.dma_start(
                                out=out_d[nt * P:(nt + 1) * P, :],
                                in_=fin_sb[:, nt, :])
                    return f

                def fin_unit_split(nt, tb):
                    def f():
                        ps = aux.tile([P, NBLK], f32, tag="aux", name="psf")
                        nc.tensor.matmul(
                            ps[:, 0:QD],
                            lhsT=o_sb[0:D, 3, nt * P:(nt + 1) * P],
                            rhs=wo_sb[0:D, 3, :],
                            start=True, stop=False,
                        )
                        nc.tensor.matmul(
                            ps[:, 0:QD],
                            lhsT=tb[0:D, (nt - 4) * P:(nt - 3) * P],
                            rhs=wo3b[:],
                            start=False, stop=True,
                        )
                        nc.vector.tensor_add(
                            fin_sb[:, nt, :], ps[:, 0:QD], fin_sb[:, nt, :])
                        nc.sync.dma_start(
                            out=out_d[nt * P:(nt + 1) * P, :],
                            in_=fin_sb[:, nt, :])
                    return f

                # before attention: qT/kT slice 0 and the first v tiles;
                # the remaining v tiles drip in with a margin
                proj_unit_q(0, 0)()
                proj_unit_q(0, 1)()
                for off, w in mchunks:
                    proj_unit_k(0, off, w)()
                for mt in range(min(2, nmt)):
                    v_unit(mt)()
                for mt in range(2, nmt):
                    pending.append((True, v_unit(mt)))

                # ---- attention, one head-pair (2p, 2p+1) at a time ----
                for p in range(4):
                    # projection for this pair must be emitted before its
                    # first score matmul: flush any backlog (not at p=0,
                    # where pending holds v tiles consumed with a margin)
                    if p > 0:
                        for _, f in pending:
                            f()
                        pending = []
                    if p < 3:
                        for nh in range(2):
                            pending.append((False, proj_unit_q(p + 1, nh)))
                        for off, w in mchunks:
                            pending.append((False, proj_unit_k(p + 1, off, w)))
                    for nb in range(2):
                        nsl = slice(nb * NBLK, (nb + 1) * NBLK)
                        oa = ops.tile([P, NBLK], f32, tag="oa")
                        ob = ops.tile([P, NBLK], f32, tag="ob")
                        pts = {}

                        def pv(mt):
                            pt = pts.pop(mt)
                            nc.tensor.matmul(
                                oa[0:D + 1, :],
                                lhsT=v_sb[:, mt, 2 * p, :],
                                rhs=pt[:, 0:NBLK],
                                start=(mt == 0), stop=(mt == nmt - 1),
                            )
                            nc.tensor.matmul(
                                ob[0:D + 1, :],
                                lhsT=v_sb[:, mt, 2 * p + 1, :],
                                rhs=pt[:, NBLK:2 * NBLK],
                                start=(mt == 0), stop=(mt == nmt - 1),
                            )

                        for mt in range(nmt):
                            sp = sps.tile([P, 2 * NBLK], f32, tag="s")
                            msl = slice(mt * P, (mt + 1) * P)
                            nc.tensor.matmul(
                                sp[:, 0:NBLK],
                                lhsT=kT_sb[0:64, p, msl],
                                rhs=qT_sb[0:64, p, nsl],
                                start=True, stop=True,
                            )
                            nc.tensor.matmul(
                                sp[:, NBLK:2 * NBLK],
                                lhsT=kT_sb[64:128, p, msl],
                                rhs=qT_sb[64:128, p, nsl],
                                start=True, stop=True,
                            )
                            pt = ppool.tile([P, 2 * NBLK], bf16, tag="pt")
                            nc.scalar.activation(
                                out=pt[:], in_=sp[:], func=EXP,
                                bias=mb_sb[:, mt:mt + 1], scale=SCALE,
                            )
                            pts[mt] = pt
                            # PV lags the scores by two m-tiles: its exp
                            # input is then always ready, so the in-order
                            # PE queue never stalls on the ACT stream, and
                            # the previous block's accumulators have time
                            # to drain before they are reused
                            if mt >= 2:
                                pv(mt - 2)
                            if pending and (
                                    p > 0 or nb == 1 or pending[0][0]):
                                pending.pop(0)[1]()
                        pv(nmt - 2)
                        pv(nmt - 1)

                        rawa = rawp.tile([P, NBLK], f32, tag="rawa")
                        rawb = rawp.tile([P, NBLK], f32, tag="rawb")
                        bcb = bcp.tile([D, 2, NBLK], f32, tag="bcb")
                        tb = rawp.tile([D, NBLK], bf16, tag="tb")
                        if not (p == 3 and nb == 1):
                            # normalize: bounce the raw denominator rows
                            # through DRAM for the partition broadcast,
                            # reciprocal out of place, then multiply
                            rcb = bcp.tile([D, 2, NBLK], f32, tag="rcb")
                            scr = dscr.tile([2, NBLK], f32, tag="scr")
                            nc.vector.tensor_copy(rawa[0:D + 1, :],
                                                  oa[0:D + 1, :])
                            nc.vector.tensor_copy(rawb[0:D + 1, :],
                                                  ob[0:D + 1, :])
                            for i, raw in ((0, rawa), (1, rawb)):
                                nc.sync.dma_start(out=scr[i:i + 1, :],
                                                  in_=raw[D:D + 1, :])
                                src = scr[i:i + 1, :]
                                bsrc = bass.AP(tensor=src.tensor,
                                               offset=src.offset,
                                               ap=[[0, D]] + src.ap[1:])
                                nc.sync.dma_start(out=rcb[0:D, i, :],
                                                  in_=bsrc)
                            nc.vector.reciprocal_approx_fast(
                                out=bcb[0:D, :, :], in_=rcb[0:D, :, :])
                            nc.vector.tensor_mul(
                                o_sb[0:D, p, nsl], rawa[0:D, :],
                                bcb[0:D, 0, :])
                            nc.vector.tensor_mul(
                                tb[0:D, :], rawb[0:D, :], bcb[0:D, 1, :])
                            nc.sync.dma_start(out=o_sb[D:P, p, nsl],
                                              in_=tb[0:D, :])
                            for nt in range(nb * 4, nb * 4 + 4):
                                pending.append((False, fin_unit(p, nt)))
                        else:
                            # last block: latency-optimized drain. ACT does
                            # the PSUM reads (its exp stream just ended),
                            # the idle PE broadcasts the denominator rows,
                            # and the output projection takes the second
                            # head half straight from tb via a split-K
                            # accumulation, skipping the shift DMA.
                            CPY = mybir.ActivationFunctionType.Copy
                            den = bcp.tile([P, 2, NBLK], bf16, tag="den")
                            nc.scalar.activation(out=den[D:D + 1, 0, :],
                                                 in_=oa[D:D + 1, :], func=CPY)
                            nc.scalar.activation(out=den[D:D + 1, 1, :],
                                                 in_=ob[D:D + 1, :], func=CPY)
                            nc.scalar.activation(out=rawa[0:D, :],
                                                 in_=oa[0:D, :], func=CPY)
                            nc.scalar.activation(out=rawb[0:D, :],
                                                 in_=ob[0:D, :], func=CPY)
                            bca = aux.tile([P, NBLK], f32, tag="aux",
                                           name="bca")
                            bcq = aux.tile([P, NBLK], f32, tag="aux",
                                           name="bcq")
                            nc.tensor.matmul(
                                bca[0:D, :], lhsT=ones_sb[D:D + 1, :],
                                rhs=den[D:D + 1, 0, :],
                                start=True, stop=True)
                            nc.tensor.matmul(
                                bcq[0:D, :], lhsT=ones_sb[D:D + 1, :],
                                rhs=den[D:D + 1, 1, :],
                                start=True, stop=True)
                            nc.vector.reciprocal_approx_fast(
                                out=bcb[0:D, 0, :], in_=bca[0:D, :])
                            nc.vector.reciprocal_approx_fast(
                                out=bcb[0:D, 1, :], in_=bcq[0:D, :])
                            nc.vector.tensor_mul(
                                o_sb[0:D, p, nsl], rawa[0:D, :],
                                bcb[0:D, 0, :])
                            nc.vector.tensor_mul(
                                tb[0:D, :], rawb[0:D, :], bcb[0:D, 1, :])
                            for nt in range(4, 8):
                                pending.append(
                                    (False, fin_unit_split(nt, tb)))
                # drain any remaining aux work (last pair's output proj)
                for _, f in pending:
                    f()

    nc.compile()
    return nc


def get_nc(nmt=None):
    if nmt is None:
        nmt = _CACHE.get("last_nmt", M // P)
    if ("nc", nmt) not in _CACHE:
        _CACHE[("nc", nmt)] = _build_nc(nmt)
    _CACHE["last_nmt"] = nmt
    return _CACHE[("nc", nmt)]


def make_in_maps(x, context, mask, Wq, Wkv, Wo, bo):
    """CPU glue: shard, transpose, cast, and compact keys by mask."""
    bf = ml_dtypes.bfloat16
    Wk = np.ascontiguousarray(Wkv[:, :INNER]).astype(bf)
    Wv = np.ascontiguousarray(Wkv[:, INNER:]).astype(bf)
    Wq_b = np.ascontiguousarray(Wq).astype(bf)
    Wo_b = np.ascontiguousarray(Wo).astype(bf)
    bo_f = np.ascontiguousarray(bo, dtype=np.float32).reshape(1, QD)

    idxs = [np.where(mask[b])[0] for b in range(B)]
    maxc = max(1, max(len(i) for i in idxs))
    nmt = (maxc + P - 1) // P
    mpad = nmt * P

    in_maps = []
    for c in range(8):
        b, s = c // 2, c % 2
        idx = idxs[b]
        cnt = len(idx)
        ctx_c = np.zeros((mpad, QD), dtype=np.float32)
        ctx_c[:cnt] = context[b][idx]
        mb = np.full(mpad, MASK_NEG, dtype=np.float32)
        mb[:cnt] = 0.0
        xT = np.ascontiguousarray(
            x[b, s * NCORE:(s + 1) * NCORE, :].T).astype(bf)
        ctxT = np.ascontiguousarray(ctx_c.T).astype(bf)
        mbt = np.ascontiguousarray(mb.reshape(nmt, P).T)
        in_maps.append({
            "xT": xT.reshape(4, P, NCORE),
            "ctxT": ctxT.reshape(4, P, mpad),
            "wq": Wq_b.reshape(4, P, INNER),
            "wk": Wk.reshape(4, P, INNER),
            "wv": Wv.reshape(4, P, INNER),
            "wo": Wo_b.reshape(4, P, QD),
            "bo": bo_f, "mb": mbt,
        })
    return in_maps, nmt


def assemble(results):
    out = np.empty((B, N, QD), dtype=np.float32)
    for c in range(8):
        b, s = c // 2, c % 2
        out[b, s * NCORE:(s + 1) * NCORE, :] = results[c]["out"]
    return out


def kernel(x, context, mask, Wq, Wkv, Wo, bo):
    from concourse.bass_utils import run_bass_kernel_spmd

    x = np.asarray(x, dtype=np.float32)
    context = np.asarray(context, dtype=np.float32)
    mask = np.asarray(mask)
    in_maps, nmt = make_in_maps(x, context, mask,
                                np.asarray(Wq, dtype=np.float32),
                                np.asarray(Wkv, dtype=np.float32),
                                np.asarray(Wo, dtype=np.float32),
                                np.asarray(bo, dtype=np.float32))
    nc = get_nc(nmt)
    res = run_bass_kernel_spmd(nc, in_maps, list(range(8)))
    return assemble(res.results)


# revision 41
# speedup vs baseline: 1.1694x; 1.1694x over previous
"""Trainium2 Bass kernel for nn_Attention_3375844294750.

Cross-attention (q from x, k/v from context) with key mask, 8 heads, d=64.
  B=4, N=M=2048, query_dim=context_dim=512, inner=512.

Sharding: 8 NeuronCores = (batch b = core//2) x (query-half = core%2).
Each core computes attention for its 1024 queries over its batch's keys.
No collectives needed (outputs are disjoint).

Key compaction: masked keys contribute exactly 0 to masked softmax, so the
CPU glue gathers only the unmasked keys (~50% of 2048) per batch, padded
to a multiple of 128; padding slots are killed by the exp bias. This
halves the score/exp/PV work.

Structure: the attention inner loop streams score->exp->PV per m-tile,
with the PV stream lagged two m-tiles behind the scores so the in-order
PE queue never waits on the ACT(exp) stream and the previous block's PSUM
accumulators have time to drain before reuse. Projection and
output-projection units are dripped into the PE slack under the exp
stream, one unit per iteration.

Per-core math (all matmuls bf16 with fp32 PSUM accumulation):
  qT = (x @ Wq)^T        [inner, n]   via rhs = x^T (CPU pre-transposed)
  kT = (ctx_c @ Wk)^T    [inner, m_c]
  v  = ctx_c @ Wv        [m_c, inner] (+ ones column per head for the
                                       softmax denominator)
  S^T = kT_h-blocks @ qT_h            [m_c, n] per head-pair, K=64
  P^T = exp(S*scale + pad_bias)       one-pass softmax (logits bounded,
                                      no max subtraction needed)
  O^T_h (+denom row) = [V_h|1]^T @ P^T_h   accumulated over m-tiles
  O_norm^T = O^T * (1/denom): the denominator rows are partition-broadcast
  via a DRAM bounce (hidden under the next block) except for the final
  block, which instead casts them to bf16 and broadcasts on the then-idle
  PE, and whose output projection takes the second head half directly
  from the normalize result via a split-K accumulation (no shift DMA) —
  keeping the drain chain short.
  out = O_norm^T-blocks^T @ Wo + bo   (SBUF-accumulated per head-pair)

Startup: inputs load as a few batched multi-dim DMAs split across the two
hardware DGE queues (sync + scalar), staged so the first projection units
wait on the least data; throwaway matmuls ramp the PE clock and a
throwaway exp pulls the activation-table load off the critical path while
the first tiles land.
"""
import os
import sys

for _p in ("/opt/trn_rl_repo", "/root/.axon_site/_ro/trn_rl_repo"):
    if os.path.isdir(_p) and _p not in sys.path:
        sys.path.insert(0, _p)
        break

import numpy as np
import ml_dtypes

B, N, M = 4, 2048, 2048
QD = 512          # query_dim == context_dim
H, D = 8, 64
INNER = H * D     # 512
SCALE = D ** -0.5
NCORE = N // 2    # queries per core = 1024
P = 128
NBLK = 512        # n-block (one PSUM bank per matmul)
MASK_NEG = -1e30

_CACHE = {}


def _build_nc(nmt):
    """Build + compile the SPMD program for nmt m-tiles (m_pad = 128*nmt)."""
    import concourse.mybir as mybir
    from concourse import bacc
    from concourse.tile import TileContext
    import concourse.bass as bass

    mpad = nmt * P
    dt = mybir.dt
    nc = bacc.Bacc("TRN2", target_bir_lowering=False, debug=False, num_devices=8)

    xT_d = nc.declare_dram_parameter("xT", [4, P, NCORE], dt.bfloat16, isOutput=False)
    ctxT_d = nc.declare_dram_parameter("ctxT", [4, P, mpad], dt.bfloat16, isOutput=False)
    wq_d = nc.declare_dram_parameter("wq", [4, P, INNER], dt.bfloat16, isOutput=False)
    wk_d = nc.declare_dram_parameter("wk", [4, P, INNER], dt.bfloat16, isOutput=False)
    wv_d = nc.declare_dram_parameter("wv", [4, P, INNER], dt.bfloat16, isOutput=False)
    wo_d = nc.declare_dram_parameter("wo", [4, P, QD], dt.bfloat16, isOutput=False)
    bo_d = nc.declare_dram_parameter("bo", [1, QD], dt.float32, isOutput=False)
    mb_d = nc.declare_dram_parameter("mb", [P, nmt], dt.float32, isOutput=False)
    out_d = nc.declare_dram_parameter("out", [NCORE, QD], dt.float32, isOutput=True)

    f32 = dt.float32
    bf16 = dt.bfloat16
    EXP = mybir.ActivationFunctionType.Exp

    with TileContext(nc) as tc:
        from contextlib import ExitStack

        with ExitStack() as ctx:
            const = ctx.enter_context(tc.tile_pool(name="const", bufs=1))

            # ---- persistent SBUF tensors ----
            wq_sb = const.tile([P, 4, INNER], bf16, tag="wq")
            xT_sb = const.tile([P, 4, NCORE], bf16, tag="xT")
            wk_sb = const.tile([P, 4, INNER], bf16, tag="wk")
            ctxT_sb = const.tile([P, 4, mpad], bf16, tag="cT")
            wv_sb = const.tile([P, 4, INNER], bf16, tag="wv")
            wo_sb = const.tile([P, 4, QD], bf16, tag="wo")
            bo_bc = const.tile([P, QD], f32, tag="bo")
            mb_sb = const.tile([P, nmt], f32, tag="mb")
            wm_sb = const.tile([P, NBLK], bf16, tag="wm")

            ones_sb = const.tile([P, D], bf16, tag="ones")
            dm_sb = const.tile([P, 8], bf16, tag="dm")
            wo3b = const.tile([D, QD], bf16, tag="wo3b")
            qT_sb = const.tile([P, 4, NCORE], bf16, tag="qT")
            kT_sb = const.tile([P, 4, mpad], bf16, tag="kT")
            v_sb = const.tile([P, nmt, H, D + 1], bf16, tag="v")
            o_sb = const.tile([P, 4, NCORE], bf16, tag="oT")
            fin_sb = const.tile([P, NCORE // P, QD], f32, tag="fin")

            # warm-up weights for the PE clock ramp (never read by output)
            nc.vector.memset(wm_sb[:], 1.0)
            nc.vector.memset(ones_sb[:], 1.0)

            # ---- input loads: batched multi-dim DMAs over two issue
            # queues, staged so the first projection units wait on the
            # least possible data (slice 0 + left column halves first)
            def dbox(t, s0, ns, p0, np_, c0, nc_):
                a = t.ap()
                S, Pp, C = a.ap[0][0], a.ap[1][0], a.ap[2][0]
                return bass.AP(
                    tensor=a.tensor,
                    offset=a.offset + s0 * S + p0 * Pp + c0 * C,
                    ap=[[Pp, np_], [S, ns], [C, nc_]])

            nc.sync.dma_start(out=wq_sb[:, 0:1, :],
                              in_=dbox(wq_d, 0, 1, 0, P, 0, INNER))
            nc.sync.dma_start(out=xT_sb[:, 0:1, 0:NBLK],
                              in_=dbox(xT_d, 0, 1, 0, P, 0, NBLK))
            nc.scalar.dma_start(out=ctxT_sb[:, :, 0:NBLK],
                                in_=dbox(ctxT_d, 0, 4, 0, P, 0, NBLK))
            nc.sync.dma_start(out=wq_sb[:, 1:4, :],
                              in_=dbox(wq_d, 1, 3, 0, P, 0, INNER))
            nc.sync.dma_start(out=xT_sb[:, 1:4, 0:NBLK],
                              in_=dbox(xT_d, 1, 3, 0, P, 0, NBLK))
            nc.scalar.dma_start(out=xT_sb[:, :, NBLK:NCORE],
                                in_=dbox(xT_d, 0, 4, 0, P, NBLK, NCORE - NBLK))
            nc.sync.dma_start(out=wk_sb[:],
                              in_=dbox(wk_d, 0, 4, 0, P, 0, INNER))
            nc.scalar.dma_start(out=ctxT_sb[:, :, NBLK:mpad],
                                in_=dbox(ctxT_d, 0, 4, 0, P, NBLK, mpad - NBLK))
            nc.sync.dma_start(out=wv_sb[:],
                              in_=dbox(wv_d, 0, 4, 0, P, 0, INNER))
            nc.scalar.dma_start(out=wo_sb[:],
                                in_=dbox(wo_d, 0, 4, 0, P, 0, QD))
            nc.sync.dma_start(out=mb_sb[:], in_=mb_d[:])
            nc.sync.dma_start(out=wo3b[:], in_=dbox(wo_d, 3, 1, D, D, 0, QD))
            bo_src = bass.AP(tensor=bo_d.ap().tensor, offset=bo_d.ap().offset,
                             ap=[[0, P]] + bo_d.ap().ap[1:])
            nc.sync.dma_start(out=bo_bc[:], in_=bo_src)

            # ones columns for the denominator trick (copies below leave them)
            nc.vector.memset(v_sb[:, :, :, D], 1.0)

            mchunks = []
            off = 0
            while off < mpad:
                w = min(NBLK, mpad - off)
                mchunks.append((off, w))
                off += w

            with tc.tile_pool(name="aux", bufs=2, space="PSUM") as aux, \
                 tc.tile_pool(name="sps", bufs=2, space="PSUM") as sps, \
                 tc.tile_pool(name="ops", bufs=1, space="PSUM") as ops, \
                 tc.tile_pool(name="ppool", bufs=6) as ppool, \
                 tc.tile_pool(name="raw", bufs=4) as rawp, \
                 tc.tile_pool(name="bcp", bufs=2) as bcp, \
                 tc.tile_pool(name="dscr", bufs=4, space="DRAM") as dscr:

                # a few throwaway matmuls ramp the PE p-state while the
                # first input tiles are still in flight, and a throwaway
                # exp pulls the activation table load off the critical path
                for _ in range(4):
                    pw = aux.tile([P, NBLK], f32, tag="aux", name="warm")
                    nc.tensor.matmul(pw[:], lhsT=wm_sb[:, 0:P],
                                     rhs=wm_sb[:], start=True, stop=True)
                nc.scalar.activation(out=dm_sb[0:1, :], in_=wm_sb[0:1, 0:8],
                                     func=EXP, scale=1.0)

                def v_unit(mt):
                    def f():
                        ps = aux.tile([P, INNER], f32, tag="aux", name="psv")
                        for kq in range(4):
                            nc.tensor.matmul(
                                ps[:],
                                lhsT=ctxT_sb[:, kq, mt * P:(mt + 1) * P],
                                rhs=wv_sb[:, kq, :],
                                start=(kq == 0), stop=(kq == 3),
                            )
                        psh = ps.rearrange("p (h d) -> p h d", h=H)
                        nc.vector.tensor_copy(v_sb[:, mt, :, 0:D], psh[:])
                    return f

                # deferred aux work (projection slices, output-proj
                # units) dripped into the attention stream so the exp
                # pipeline never starves
                pending = []

                def proj_unit_q(mi, nh):
                    def f():
                        ps = aux.tile([P, NBLK], f32, tag="aux", name="psq")
                        for kq in range(4):
                            nc.tensor.matmul(
                                ps[:],
                                lhsT=wq_sb[:, kq, mi * P:(mi + 1) * P],
                                rhs=xT_sb[:, kq, nh * NBLK:(nh + 1) * NBLK],
                                start=(kq == 0), stop=(kq == 3),
                            )
                        nc.vector.tensor_copy(
                            qT_sb[:, mi, nh * NBLK:(nh + 1) * NBLK], ps[:])
                    return f

                def proj_unit_k(mi, off, w):
                    def f():
                        ps = aux.tile([P, NBLK], f32, tag="aux", name="psk")
                        for kq in range(4):
                            nc.tensor.matmul(
                                ps[:, 0:w],
                                lhsT=wk_sb[:, kq, mi * P:(mi + 1) * P],
                                rhs=ctxT_sb[:, kq, off:off + w],
                                start=(kq == 0), stop=(kq == 3),
                            )
                        nc.vector.tensor_copy(
                            kT_sb[:, mi, off:off + w], ps[:, 0:w])
                    return f

                def fin_unit(p, nt):
                    def f():
                        ps = aux.tile([P, NBLK], f32, tag="aux", name="psf")
                        nc.tensor.matmul(
                            ps[:, 0:QD],
                            lhsT=o_sb[:, p, nt * P:(nt + 1) * P],
                            rhs=wo_sb[:, p, :],
                            start=True, stop=True,
                        )
                        if p == 0:
                            nc.vector.tensor_add(
                                fin_sb[:, nt, :], ps[:, 0:QD], bo_bc[:])
                        else:
                            nc.vector.tensor_add(
                                fin_sb[:, nt, :], ps[:, 0:QD],
                                fin_sb[:, nt, :])
                        if p == 3:
                            nc.sync.dma_start(
                                out=out_d[nt * P:(nt + 1) * P, :],
                                in_=fin_sb[:, nt, :])
                    return f

                def fin_unit_split(nt, tb):
                    def f():
                        ps = aux.tile([P, NBLK], f32, tag="aux", name="psf")
                        nc.tensor.matmul(
                            ps[:, 0:QD],
                            lhsT=o_sb[0:D, 3, nt * P:(nt + 1) * P],
                            rhs=wo_sb[0:D, 3, :],
                            start=True, stop=False,
                        )
                        nc.tensor.matmul(
                            ps[:, 0:QD],
                            lhsT=tb[0:D, (nt - 4) * P:(nt - 3) * P],
                            rhs=wo3b[:],
                            start=False, stop=True,
                        )
                        nc.vector.tensor_add(
                            fin_sb[:, nt, :], ps[:, 0:QD], fin_sb[:, nt, :])
                        nc.sync.dma_start(
                            out=out_d[nt * P:(nt + 1) * P, :],
                            in_=fin_sb[:, nt, :])
                    return f

                # before attention: qT/kT slice 0 and the first v tiles;
                # the remaining v tiles drip in with a margin
                proj_unit_q(0, 0)()
                proj_unit_q(0, 1)()
                for off, w in mchunks:
                    proj_unit_k(0, off, w)()
                for mt in range(min(2, nmt)):
                    v_unit(mt)()
                for mt in range(2, nmt):
                    pending.append((True, v_unit(mt)))

                # ---- attention, one head-pair (2p, 2p+1) at a time ----
                for p in range(4):
                    # projection for this pair must be emitted before its
                    # first score matmul: flush any backlog (not at p=0,
                    # where pending holds v tiles consumed with a margin)
                    if p > 0:
                        for _, f in pending:
                            f()
                        pending = []
                    if p < 3:
                        for nh in range(2):
                            pending.append((False, proj_unit_q(p + 1, nh)))
                        for off, w in mchunks:
                            pending.append((False, proj_unit_k(p + 1, off, w)))
                    for nb in range(2):
                        nsl = slice(nb * NBLK, (nb + 1) * NBLK)
                        oa = ops.tile([P, NBLK], f32, tag="oa")
                        ob = ops.tile([P, NBLK], f32, tag="ob")
                        pts = {}

                        def pv(mt):
                            pt = pts.pop(mt)
                            nc.tensor.matmul(
                                oa[0:D + 1, :],
                                lhsT=v_sb[:, mt, 2 * p, :],
                                rhs=pt[:, 0:NBLK],
                                start=(mt == 0), stop=(mt == nmt - 1),
                            )
                            nc.tensor.matmul(
                                ob[0:D + 1, :],
                                lhsT=v_sb[:, mt, 2 * p + 1, :],
                                rhs=pt[:, NBLK:2 * NBLK],
                                start=(mt == 0), stop=(mt == nmt - 1),
                            )

                        for mt in range(nmt):
                            sp = sps.tile([P, 2 * NBLK], f32, tag="s")
                            msl = slice(mt * P, (mt + 1) * P)
                            nc.tensor.matmul(
                                sp[:, 0:NBLK],
                                lhsT=kT_sb[0:64, p, msl],
                                rhs=qT_sb[0:64, p, nsl],
                                start=True, stop=True,
                            )
                            nc.tensor.matmul(
                                sp[:, NBLK:2 * NBLK],
                                lhsT=kT_sb[64:128, p, msl],
                                rhs=qT_sb[64:128, p, nsl],
                                start=True, stop=True,
                            )
                            pt = ppool.tile([P, 2 * NBLK], bf16, tag="pt")
                            nc.scalar.activation(
                                out=pt[:], in_=sp[:], func=EXP,
                                bias=mb_sb[:, mt:mt + 1], scale=SCALE,
                            )
                            pts[mt] = pt
                            # PV lags the scores by two m-tiles: its exp
                            # input is then always ready, so the in-order
                            # PE queue never stalls on the ACT stream, and
                            # the previous block's accumulators have time
                            # to drain before they are reused
                            if mt >= 2:
                                pv(mt - 2)
                            if pending and (
                                    p > 0 or nb == 1 or pending[0][0]):
                                pending.pop(0)[1]()
                        if nmt >= 2:
                            pv(nmt - 2)
                        pv(nmt - 1)

                        rawa = rawp.tile([P, NBLK], f32, tag="rawa")
                        rawb = rawp.tile([P, NBLK], f32, tag="rawb")
                        bcb = bcp.tile([D, 2, NBLK], f32, tag="bcb")
                        tb = rawp.tile([D, NBLK], bf16, tag="tb")
                        if not (p == 3 and nb == 1):
                            # normalize: bounce the raw denominator rows
                            # through DRAM for the partition broadcast,
                            # reciprocal out of place, then multiply
                            rcb = bcp.tile([D, 2, NBLK], f32, tag="rcb")
                            scr = dscr.tile([2, NBLK], f32, tag="scr")
                            nc.vector.tensor_copy(rawa[0:D + 1, :],
                                                  oa[0:D + 1, :])
                            nc.vector.tensor_copy(rawb[0:D + 1, :],
                                                  ob[0:D + 1, :])
                            for i, raw in ((0, rawa), (1, rawb)):
                                nc.sync.dma_start(out=scr[i:i + 1, :],
                                                  in_=raw[D:D + 1, :])
                                src = scr[i:i + 1, :]
                                bsrc = bass.AP(tensor=src.tensor,
                                               offset=src.offset,
                                               ap=[[0, D]] + src.ap[1:])
                                nc.sync.dma_start(out=rcb[0:D, i, :],
                                                  in_=bsrc)
                            nc.vector.reciprocal_approx_fast(
                                out=bcb[0:D, :, :], in_=rcb[0:D, :, :])
                            nc.vector.tensor_mul(
                                o_sb[0:D, p, nsl], rawa[0:D, :],
                                bcb[0:D, 0, :])
                            nc.vector.tensor_mul(
                                tb[0:D, :], rawb[0:D, :], bcb[0:D, 1, :])
                            nc.sync.dma_start(out=o_sb[D:P, p, nsl],
                                              in_=tb[0:D, :])
                            for nt in range(nb * 4, nb * 4 + 4):
                                pending.append((False, fin_unit(p, nt)))
                        else:
                            # last block: latency-optimized drain. ACT does
                            # the PSUM reads (its exp stream just ended),
                            # the idle PE broadcasts the denominator rows,
                            # and the output projection takes the second
                            # head half straight from tb via a split-K
                            # accumulation, skipping the shift DMA.
                            CPY = mybir.ActivationFunctionType.Copy
                            den = bcp.tile([P, 2, NBLK], bf16, tag="den")
                            nc.scalar.activation(out=den[D:D + 1, 0, :],
                                                 in_=oa[D:D + 1, :], func=CPY)
                            nc.scalar.activation(out=den[D:D + 1, 1, :],
                                                 in_=ob[D:D + 1, :], func=CPY)
                            nc.scalar.activation(out=rawa[0:D, :],
                                                 in_=oa[0:D, :], func=CPY)
                            nc.scalar.activation(out=rawb[0:D, :],
                                                 in_=ob[0:D, :], func=CPY)
                            bca = aux.tile([P, NBLK], f32, tag="aux",
                                           name="bca")
                            bcq = aux.tile([P, NBLK], f32, tag="aux",
                                           name="bcq")
                            nc.tensor.matmul(
                                bca[0:D, :], lhsT=ones_sb[D:D + 1, :],
                                rhs=den[D:D + 1, 0, :],
                                start=True, stop=True)
                            nc.tensor.matmul(
                                bcq[0:D, :], lhsT=ones_sb[D:D + 1, :],
                                rhs=den[D:D + 1, 1, :],
                                start=True, stop=True)
                            nc.vector.reciprocal_approx_fast(
                                out=bcb[0:D, 0, :], in_=bca[0:D, :])
                            nc.vector.reciprocal_approx_fast(
                                out=bcb[0:D, 1, :], in_=bcq[0:D, :])
                            nc.vector.tensor_mul(
                                o_sb[0:D, p, nsl], rawa[0:D, :],
                                bcb[0:D, 0, :])
                            nc.vector.tensor_mul(
                                tb[0:D, :], rawb[0:D, :], bcb[0:D, 1, :])
                            for nt in range(4, 8):
                                pending.append(
                                    (False, fin_unit_split(nt, tb)))
                # drain any remaining aux work (last pair's output proj)
                for _, f in pending:
                    f()

    nc.compile()
    return nc


def get_nc(nmt=None):
    if nmt is None:
        nmt = _CACHE.get("last_nmt", M // P)
    if ("nc", nmt) not in _CACHE:
        _CACHE[("nc", nmt)] = _build_nc(nmt)
    _CACHE["last_nmt"] = nmt
    return _CACHE[("nc", nmt)]


def make_in_maps(x, context, mask, Wq, Wkv, Wo, bo):
    """CPU glue: shard, transpose, cast, and compact keys by mask."""
    bf = ml_dtypes.bfloat16
    Wk = np.ascontiguousarray(Wkv[:, :INNER]).astype(bf)
    Wv = np.ascontiguousarray(Wkv[:, INNER:]).astype(bf)
    Wq_b = np.ascontiguousarray(Wq).astype(bf)
    Wo_b = np.ascontiguousarray(Wo).astype(bf)
    bo_f = np.ascontiguousarray(bo, dtype=np.float32).reshape(1, QD)

    idxs = [np.where(mask[b])[0] for b in range(B)]
    maxc = max(1, max(len(i) for i in idxs))
    nmt = (maxc + P - 1) // P
    mpad = nmt * P

    in_maps = []
    for c in range(8):
        b, s = c // 2, c % 2
        idx = idxs[b]
        cnt = len(idx)
        ctx_c = np.zeros((mpad, QD), dtype=np.float32)
        ctx_c[:cnt] = context[b][idx]
        mb = np.full(mpad, MASK_NEG, dtype=np.float32)
        mb[:cnt] = 0.0
        xT = np.ascontiguousarray(
            x[b, s * NCORE:(s + 1) * NCORE, :].T).astype(bf)
        ctxT = np.ascontiguousarray(ctx_c.T).astype(bf)
        mbt = np.ascontiguousarray(mb.reshape(nmt, P).T)
        in_maps.append({
            "xT": xT.reshape(4, P, NCORE),
            "ctxT": ctxT.reshape(4, P, mpad),
            "wq": Wq_b.reshape(4, P, INNER),
            "wk": Wk.reshape(4, P, INNER),
            "wv": Wv.reshape(4, P, INNER),
            "wo": Wo_b.reshape(4, P, QD),
            "bo": bo_f, "mb": mbt,
        })
    return in_maps, nmt


def assemble(results):
    out = np.empty((B, N, QD), dtype=np.float32)
    for c in range(8):
        b, s = c // 2, c % 2
        out[b, s * NCORE:(s + 1) * NCORE, :] = results[c]["out"]
    return out


def kernel(x, context, mask, Wq, Wkv, Wo, bo):
    from concourse.bass_utils import run_bass_kernel_spmd

    x = np.asarray(x, dtype=np.float32)
    context = np.asarray(context, dtype=np.float32)
    mask = np.asarray(mask)
    in_maps, nmt = make_in_maps(x, context, mask,
                                np.asarray(Wq, dtype=np.float32),
                                np.asarray(Wkv, dtype=np.float32),
                                np.asarray(Wo, dtype=np.float32),
                                np.asarray(bo, dtype=np.float32))
    nc = get_nc(nmt)
    res = run_bass_kernel_spmd(nc, in_maps, list(range(8)))
    return assemble(res.results)
